# revision 18
# baseline (speedup 1.0000x reference)
"""Trainium2 Bass kernel for nn_GAT_42786464203341.

8-way tensor parallel (Megatron-style) over one trn2 chip:
  - The GAT edges are block-diagonal fully-connected per sample, so message
    passing is dense per-sample attention with scores leaky(el[i] + er[j]),
    softmaxed over source i.
  - Activations feature-major (x^T: [D, nodes]); all matmuls bf16 (FWL
    weight loads, free-size-unconstrained); PSUM accumulates f32.
  - LayerNorm is folded into the weights: W~ = diag(g) W on the host, the
    -mu and +bias corrections enter each matmul as one rank-2 accumulation
    step (lhsT = [c; u], rhs = [-mu_row; sigma_row]), and the per-node
    1/sigma scale is applied at PSUM eviction (ACT per-partition scale for
    node-major outputs, DVE multiply by a broadcast row otherwise).  The
    explicit normalize / broadcast / affine chains of a direct LN are gone.
  - Attention is head-parallel (2 heads/core); W_proj row-sharded ->
    partial [D, nodes] -> AllReduce (fp16).  FFN column/row sharded ->
    AllReduce.  Head vocab-sharded; host concatenates the 8 logits slices.
"""

import time
from contextlib import ExitStack

import ml_dtypes
import numpy as np

import concourse.bass as bass
import concourse.tile as tile
from concourse import bacc, mybir
from concourse.masks import make_identity

F32 = mybir.dt.float32
F16 = mybir.dt.float16
BF16 = mybir.dt.bfloat16

B, T, NOBJ = 2, 265, 9
D, H, DH = 1536, 16, 96
V, PV, L, FF = 8192, 512, 3, 6144
N = B * T          # 530
NC = 8             # cores
HPC = H // NC      # heads per core
FFL = FF // NC     # 768
VL = V // NC       # 1024
NCH = T + 1        # 266 (col 265 of each chunk is zero padding)
NP = B * NCH       # 532
KD = D // 128      # 12
KF = FFL // 128    # 6
AC = 200           # folded attention weight cols: 192 Wh | 2 el | 2 er | 2 leaky | 2 pad
MT = [(0, 128), (128, 128), (256, 10)]   # node tiles per batch (start, size)
MT_REAL = [128, 128, 9]                  # non-pad rows per node tile
EPS = 1e-5

_CACHE = {}


# --------------------------------------------------------------------------
# host-side input prep
# --------------------------------------------------------------------------

def _block_diag_edges_np():
    base = np.arange(T)
    src = np.concatenate([g * T + np.repeat(base, T) for g in range(B)])
    dst = np.concatenate([g * T + np.tile(base, T) for g in range(B)])
    return src.astype(np.int64), dst.astype(np.int64)


def _host_inputs(inp):
    f32 = np.float32
    bf16 = ml_dtypes.bfloat16
    objs_e = np.asarray(inp["obj_emb_w"])[np.asarray(inp["objs"])]
    pe = np.asarray(inp["poss_emb_w"])[np.asarray(inp["poss"])]
    nfeat = np.concatenate([objs_e, pe[:, :NOBJ], pe[:, NOBJ:]], axis=-1)
    z = np.asarray(inp["tok_emb"])[np.asarray(inp["z_indices"])]
    x0 = np.concatenate([nfeat, z], axis=1) + np.asarray(inp["pos_emb"])[:, :T]
    x0 = x0.reshape(N, D).astype(f32)

    x0t = np.zeros((D, NP), f32)
    for b in range(B):
        x0t[:, b * NCH:b * NCH + T] = x0[b * T:(b + 1) * T].T

    W_attn = np.asarray(inp["W_attn"], f32)
    a_l = np.asarray(inp["a_l"], f32)
    a_r = np.asarray(inp["a_r"], f32)
    W_proj = np.asarray(inp["W_proj"], f32)
    W_fc = np.asarray(inp["W_fc"], f32)
    W_out = np.asarray(inp["W_out"], f32)
    head_w = np.asarray(inp["head_w"], f32)
    ln1_g = np.asarray(inp["ln1_g"], f32)
    ln1_b = np.asarray(inp["ln1_b"], f32)
    ln2_g = np.asarray(inp["ln2_g"], f32)
    ln2_b = np.asarray(inp["ln2_b"], f32)
    lnf_g = np.asarray(inp["lnf_g"], f32)
    lnf_b = np.asarray(inp["lnf_b"], f32)
    b_fc = np.asarray(inp["b_fc"], f32)

    def cols(vec, k_tiles):  # [D'] -> [128, k_tiles]
        return np.asarray(vec, f32).reshape(k_tiles, 128).T.copy()

    maps = []
    for c in range(NC):
        h0 = c * HPC
        # ---- attention: fold ln1 gain, a_l/a_r vectors -------------------
        wattn = np.zeros((L, D, AC), f32)
        uattn = np.zeros((L, AC), f32)
        for lx in range(L):
            for j in range(HPC):
                hg = h0 + j
                blk = W_attn[lx][:, hg * DH:(hg + 1) * DH]        # [D, DH]
                wel = blk @ a_l[lx, hg]                           # [D]
                wer = blk @ a_r[lx, hg]
                wattn[lx, :, j * DH:(j + 1) * DH] = ln1_g[lx][:, None] * blk
                wattn[lx, :, 192 + j] = ln1_g[lx] * wel
                wattn[lx, :, 194 + j] = ln1_g[lx] * wer
                uattn[lx, j * DH:(j + 1) * DH] = ln1_b[lx] @ blk
                uattn[lx, 192 + j] = ln1_b[lx] @ wel
                uattn[lx, 194 + j] = ln1_b[lx] @ wer
        cuattn = np.stack([wattn.sum(axis=1), uattn], axis=1)     # [L, 2, AC]

        wproj = np.stack(
            [W_proj[:, (h0 + j) * DH:(h0 + j + 1) * DH, :] for j in range(HPC)],
            axis=1,
        )                                                          # [L, HPC, DH, D]

        # ---- FFN: fold ln2 gain into W_fc, ln2 bias + b_fc into u -------
        fsl = slice(c * FFL, (c + 1) * FFL)
        wfc = ln2_g[:, :, None] * W_fc[:, :, fsl]                  # [L, D, FFL]
        ufc = np.einsum("ld,ldf->lf", ln2_b, W_fc[:, :, fsl]) + b_fc[:, fsl]
        cufc = np.stack([wfc.sum(axis=1), ufc], axis=1)            # [L, 2, FFL]

        wout = np.ascontiguousarray(W_out[:, fsl, :])              # [L, FFL, D]

        # ---- head: fold final ln ----------------------------------------
        vsl = slice(c * VL, (c + 1) * VL)
        whead = lnf_g[:, None] * head_w[:, vsl]                    # [D, VL]
        uhead = lnf_b @ head_w[:, vsl]
        cuhead = np.stack([whead.sum(axis=0), uhead], axis=0)      # [2, VL]

        maps.append({
            "x0t": x0t,
            "wattn": np.ascontiguousarray(wattn).astype(bf16),
            "cuattn": np.ascontiguousarray(cuattn).astype(bf16),
            "wproj": np.ascontiguousarray(wproj).astype(bf16),
            "wfc": np.ascontiguousarray(wfc).astype(bf16),
            "cufc": np.ascontiguousarray(cufc).astype(bf16),
            "wout": np.ascontiguousarray(wout).astype(bf16),
            "whead": np.ascontiguousarray(whead).astype(bf16),
            "cuhead": np.ascontiguousarray(cuhead).astype(bf16),
            "ones_col": np.ones((128, 1), bf16),
            "ones_row": np.ones((1, 128), bf16),
            "bout8_l": np.stack([cols(np.asarray(inp["b_out"], f32)[lx] / NC, KD)
                                 for lx in range(L)]),
            "bproj8_l": np.stack([cols(np.asarray(inp["b_proj"], f32)[lx] / NC, KD)
                                  for lx in range(L)]),
        })
    return maps


# --------------------------------------------------------------------------
# device program
# --------------------------------------------------------------------------

def _build_nc(reps=1, use_cc=True):
    nc = bacc.Bacc("TRN2", target_bir_lowering=False, debug=False, num_devices=NC)

    d_x0t = nc.declare_dram_parameter("x0t", [D, NP], F32, isOutput=False)
    d_wattn = nc.declare_dram_parameter("wattn", [L, D, AC], BF16, isOutput=False)
    d_cuattn = nc.declare_dram_parameter("cuattn", [L, 2, AC], BF16, isOutput=False)
    d_wproj = nc.declare_dram_parameter("wproj", [L, HPC, DH, D], BF16, isOutput=False)
    d_wfc = nc.declare_dram_parameter("wfc", [L, D, FFL], BF16, isOutput=False)
    d_cufc = nc.declare_dram_parameter("cufc", [L, 2, FFL], BF16, isOutput=False)
    d_wout = nc.declare_dram_parameter("wout", [L, FFL, D], BF16, isOutput=False)
    d_whead = nc.declare_dram_parameter("whead", [D, VL], BF16, isOutput=False)
    d_cuhead = nc.declare_dram_parameter("cuhead", [2, VL], BF16, isOutput=False)
    d_ones_col = nc.declare_dram_parameter("ones_col", [128, 1], BF16, isOutput=False)
    d_ones_row = nc.declare_dram_parameter("ones_row", [1, 128], BF16, isOutput=False)
    d_bout8 = nc.declare_dram_parameter("bout8_l", [L, 128, KD], F32, isOutput=False)
    d_bproj8 = nc.declare_dram_parameter("bproj8_l", [L, 128, KD], F32, isOutput=False)
    d_logits = nc.declare_dram_parameter("logits", [VL, N], F32, isOutput=True)

    ar_in, ar_out = {}, {}
    for l in range(L):
        for s in range(2):
            ar_in[l, s] = nc.dram_tensor(f"arin_{l}_{s}", [D, N], F16)
            ar_out[l, s] = nc.dram_tensor(
                f"arout_{l}_{s}", [D, N], F16, addr_space="Shared"
            )

    AF = mybir.ActivationFunctionType
    ALU = mybir.AluOpType

    with tile.TileContext(nc) as tc, ExitStack() as ctx:
        res = ctx.enter_context(tc.tile_pool(name="res", bufs=1))
        cst = ctx.enter_context(tc.tile_pool(name="cst", bufs=2))
        a1 = ctx.enter_context(tc.tile_pool(name="a1", bufs=1))
        a2 = ctx.enter_context(tc.tile_pool(name="a2", bufs=2))
        a3 = ctx.enter_context(tc.tile_pool(name="a3", bufs=3))
        wgt = ctx.enter_context(tc.tile_pool(name="wgt", bufs=1))
        ps2 = ctx.enter_context(tc.tile_pool(name="ps2", bufs=2, space="PSUM"))
        ps3 = ctx.enter_context(tc.tile_pool(name="ps3", bufs=3, space="PSUM"))

        ones_col = res.tile([128, 1], BF16, tag="ones_col")
        nc.sync.dma_start(out=ones_col[:], in_=d_ones_col[:])
        ones_row = res.tile([1, 128], BF16, tag="ones_row")
        nc.sync.dma_start(out=ones_row[:], in_=d_ones_row[:])
        ident = res.tile([128, 128], BF16, tag="ident")
        make_identity(nc, ident[:])
        identF = res.tile([128, 128], F32, tag="identF")
        make_identity(nc, identF[:])
        eps_col = res.tile([1, 1], F32, tag="eps")
        nc.vector.memset(eps_col[:], EPS)

        def stats(xb16, sq16, btag):
            """feature-major LN stats from bf16 x tiles.

            Returns (mneg16 [1, NCH] = -mu, sig16 [1, NCH] = sigma,
                     rs16 [1, NCH] = 1/sigma), all bf16."""
            p_sums = ps2.tile([1, NCH], F32, tag="row")
            for k in range(KD):
                nc.tensor.matmul(
                    p_sums[:], ones_col[:], xb16[k][:],
                    start=(k == 0), stop=(k == KD - 1),
                )
            p_sqs = ps2.tile([1, NCH], F32, tag="row")
            for k in range(KD):
                nc.tensor.matmul(
                    p_sqs[:], ones_col[:], sq16[k][:],
                    start=(k == 0), stop=(k == KD - 1),
                )
            mneg16 = a1.tile([1, NCH], BF16, name=f"mneg{btag}", tag=f"mneg{btag}")
            with nc.allow_low_precision("bf16 stats"):
                nc.vector.tensor_scalar(
                    mneg16[:], p_sums[:], -1.0 / D, None, ALU.mult
                )
            m_row = a1.tile([1, NCH], F32, tag="m_row")
            nc.vector.tensor_scalar(m_row[:], p_sums[:], 1.0 / D, None, ALU.mult)
            ms = a1.tile([1, NCH], F32, tag="ms_row")
            nc.scalar.activation(ms[:], m_row[:], AF.Square)
            var = a1.tile([1, NCH], F32, tag="var_row")
            nc.vector.scalar_tensor_tensor(
                var[:], p_sqs[:], 1.0 / D, ms[:], ALU.mult, ALU.subtract
            )
            std = a1.tile([1, NCH], F32, tag="std_row")
            nc.scalar.activation(std[:], var[:], AF.Sqrt, bias=eps_col[:])
            sig16 = a1.tile([1, NCH], BF16, name=f"sig{btag}", tag=f"sig{btag}")
            nc.scalar.copy(sig16[:], std[:])
            rs16 = a1.tile([1, NCH], BF16, name=f"rs16{btag}", tag=f"rs16{btag}")
            with nc.allow_low_precision("bf16 stats"):
                nc.vector.reciprocal(rs16[:], std[:])
            return mneg16, sig16, rs16

        def make_x16(xb, btag):
            """bf16 copies of x plus bf16 squares."""
            xb16, sq16 = [], []
            for k in range(KD):
                t = a1.tile([128, NCH], BF16, name=f"x16{btag}_{k}", tag=f"x16{btag}_{k}")
                nc.scalar.copy(t[:], xb[k][:])
                xb16.append(t)
            for k in range(KD):
                t = a2.tile([128, NCH], BF16, tag=f"sq{k % 4}")
                nc.scalar.activation(t[:], xb[k][:], AF.Square)
                sq16.append(t)
            return xb16, sq16

        def rb_bcast(rs16):
            """broadcast 1/sigma row -> [128, NCH] f32 tile."""
            p_rb = ps3.tile([128, NCH], F32, tag="bc")
            nc.tensor.matmul(p_rb[:], ones_row[:], rs16[:], start=True, stop=True)
            rb_s = a1.tile([128, NCH], F32, tag="rb_s")
            nc.scalar.copy(rb_s[:], p_rb[:])
            return rb_s

        def partial_out(b, psum, b8_sb, mi, dram):
            """part = psum + b/8 in fp16; DMA into this batch's AR columns."""
            part = a3.tile([128, NCH], F16, tag="part")
            with nc.allow_low_precision("fp16 allreduce payload"):
                nc.vector.tensor_scalar(
                    part[:], psum[:], b8_sb[:, mi:mi + 1], None, ALU.add
                )
            nc.sync.dma_start(
                out=dram[mi * 128:(mi + 1) * 128, b * T:(b + 1) * T],
                in_=part[:, 0:T],
            )

        def all_reduce(l, s):
            if use_cc:
                nc.gpsimd.collective_compute(
                    "AllReduce", ALU.add,
                    replica_groups=[list(range(NC))],
                    ins=[ar_in[l, s][:].opt()],
                    outs=[ar_out[l, s][:].opt()],
                )
            else:
                nc.gpsimd.dma_start(out=ar_out[l, s][:], in_=ar_in[l, s][:])

        def refresh_xt(xb, l, s, b):
            for k in range(KD):
                tmp = a3.tile([128, T], F16, tag="artmp")
                nc.gpsimd.dma_start(
                    out=tmp[:],
                    in_=ar_out[l, s][k * 128:(k + 1) * 128, b * T:(b + 1) * T],
                )
                nc.vector.tensor_add(xb[k][:, 0:T], xb[k][:, 0:T], tmp[:])

        def attn_sublayer(b, l, xb, wa, wp, cA, uA, bproj8_sb):
            xb16, sq16 = make_x16(xb, b)
            mneg16, sig16, rs16 = stats(xb16, sq16, b)

            # per-node-tile 1/sigma columns for ACT eviction scale (must be f32)
            rs_cols = []
            for mi, (ms, msz) in enumerate(MT):
                pt = ps2.tile([128, 1], BF16, tag="row")
                nc.tensor.transpose(
                    pt[:msz, :], rs16[:, ms:ms + msz], ident[0:1, 0:1]
                )
                rc = a1.tile([128, 1], F32, tag=f"rsc{mi}")
                nc.scalar.copy(rc[:msz, :], pt[:msz, :])
                rs_cols.append(rc)

            whsb, escs = [], []
            for mi, (ms, msz) in enumerate(MT):
                p = ps3.tile([128, AC], F32, tag="mm")
                for k in range(KD):
                    nc.tensor.matmul(
                        p[:msz, :], xb16[k][:, ms:ms + msz], wa[k][:],
                        start=(k == 0), stop=False,
                    )
                nc.tensor.matmul(
                    p[:msz, :], mneg16[:, ms:ms + msz], cA[:],
                    start=False, stop=False,
                )
                nc.tensor.matmul(
                    p[:msz, :], sig16[:, ms:ms + msz], uA[:],
                    start=False, stop=True,
                )
                w = a1.tile([128, 192], BF16, tag=f"whsb{mi}_{b}")
                nc.scalar.activation(
                    w[:msz, :], p[:msz, 0:192], AF.Copy,
                    scale=rs_cols[mi][:msz, :],
                )
                # esc cols: 0:2 el, 2:4 er, 4:6 leaky el (all f32, rs-scaled)
                esc = a1.tile([128, 8], F32, tag=f"esc{mi}_{b}")
                nc.scalar.activation(
                    esc[:msz, 0:4], p[:msz, 192:196], AF.Copy,
                    scale=rs_cols[mi][:msz, :],
                )
                nc.vector.tensor_scalar(
                    esc[:msz, 4:6], esc[:msz, 0:2], 0.2, None, ALU.mult
                )
                whsb.append(w)
                escs.append(esc)

            erow = [
                a1.tile([1, NCH], BF16, name=f"er{j}_{b}", tag=f"er{j}_{b}")
                for j in range(HPC)
            ]
            for mi, (ms, msz) in enumerate(MT):
                for j in range(HPC):
                    pt = ps2.tile([1, 128], F32, tag="row")
                    nc.tensor.transpose(
                        pt[:, :msz], escs[mi][:msz, 2 + j:3 + j],
                        identF[:msz, :msz],
                    )
                    with nc.allow_low_precision("bf16 scores"):
                        nc.scalar.copy(erow[j][:, ms:ms + msz], pt[:, :msz])

            aggt = []
            for j in range(HPC):
                p_er = ps3.tile([128, NCH], F32, tag="bc")
                nc.tensor.matmul(
                    p_er[:], ones_row[:], erow[j][:], start=True, stop=True
                )
                e_tiles = []
                for mi in range(3):
                    rsz = MT_REAL[mi]
                    e1 = a2.tile([128, NCH], BF16, tag=f"e{mi}")
                    nc.scalar.activation(
                        e1[:rsz, :], p_er[:rsz, :], AF.Exp,
                        bias=escs[mi][:rsz, j:j + 1],
                    )
                    e2 = a1.tile([128, NCH], BF16, tag="e2")
                    nc.scalar.activation(
                        e2[:rsz, :], p_er[:rsz, :], AF.Exp, scale=0.2,
                        bias=escs[mi][:rsz, 4 + j:5 + j],
                    )
                    with nc.allow_low_precision("bf16 scores"):
                        nc.vector.tensor_max(e1[:rsz, :], e1[:rsz, :], e2[:rsz, :])
                    e_tiles.append(e1)
                p_s = ps2.tile([1, NCH], F32, tag="row")
                for mi in range(3):
                    rsz = MT_REAL[mi]
                    nc.tensor.matmul(
                        p_s[:], ones_col[:rsz, :], e_tiles[mi][:rsz, :],
                        start=(mi == 0), stop=(mi == 2),
                    )
                r16 = a1.tile([1, NCH], BF16, tag="r16")
                with nc.allow_low_precision("bf16 softmax"):
                    nc.vector.reciprocal(r16[:], p_s[:])
                p_rb2 = ps3.tile([DH, NCH], F32, tag="bc")
                nc.tensor.matmul(
                    p_rb2[:], ones_row[:, :DH], r16[:], start=True, stop=True
                )
                rb_sb = a1.tile([DH, NCH], F32, tag="rb_sb")
                nc.scalar.copy(rb_sb[:], p_rb2[:])
                p_agg = ps3.tile([DH, NCH], F32, tag="mm")
                for mi in range(3):
                    rsz = MT_REAL[mi]
                    nc.tensor.matmul(
                        p_agg[:],
                        whsb[mi][:rsz, j * DH:(j + 1) * DH],
                        e_tiles[mi][:rsz, :],
                        start=(mi == 0), stop=(mi == 2),
                    )
                at = a1.tile([DH, NCH], BF16, tag=f"aggt{j}_{b}")
                with nc.allow_low_precision("bf16 agg"):
                    nc.vector.tensor_mul(at[:], p_agg[:], rb_sb[:])
                aggt.append(at)

            for mi in range(KD):
                p = ps3.tile([128, NCH], F32, tag="mm")
                for j in range(HPC):
                    nc.tensor.matmul(
                        p[:], wp[j][:, mi * 128:(mi + 1) * 128], aggt[j][:],
                        start=(j == 0), stop=(j == HPC - 1),
                    )
                partial_out(b, p, bproj8_sb, mi, ar_in[l, 0])

        def ffn_sublayer(b, l, xb, wfc_sb, wout_sb, cF, uF, bout8_sb):
            xb16, sq16 = make_x16(xb, b)
            mneg16, sig16, rs16 = stats(xb16, sq16, b)
            rb_s = rb_bcast(rs16)

            g_tiles = []
            for mi in range(KF):
                p = ps3.tile([128, NCH], F32, tag="mm")
                for k in range(KD):
                    nc.tensor.matmul(
                        p[:], wfc_sb[k][:, mi * 128:(mi + 1) * 128], xb16[k][:],
                        start=(k == 0), stop=False,
                    )
                nc.tensor.matmul(
                    p[:], cF[:, mi * 128:(mi + 1) * 128], mneg16[:],
                    start=False, stop=False,
                )
                nc.tensor.matmul(
                    p[:], uF[:, mi * 128:(mi + 1) * 128], sig16[:],
                    start=False, stop=True,
                )
                tmp = a2.tile([128, NCH], F32, tag="fftmp")
                nc.vector.tensor_mul(tmp[:], p[:], rb_s[:])
                g = a2.tile([128, NCH], BF16, tag=f"g{mi}")
                nc.scalar.activation(g[:], tmp[:], AF.Gelu)
                g_tiles.append(g)
            for mi in range(KD):
                p = ps3.tile([128, NCH], F32, tag="mm")
                for k in range(KF):
                    nc.tensor.matmul(
                        p[:], wout_sb[k][:, mi * 128:(mi + 1) * 128],
                        g_tiles[k][:],
                        start=(k == 0), stop=(k == KF - 1),
                    )
                partial_out(b, p, bout8_sb, mi, ar_in[l, 1])

        for _rep in range(reps):
            xtb = []
            for b in range(B):
                row = []
                for k in range(KD):
                    t = res.tile([128, NCH], F32, name=f"xt{b}_{k}", tag=f"xt{b}_{k}")
                    nc.sync.dma_start(
                        out=t[:],
                        in_=d_x0t[k * 128:(k + 1) * 128, b * NCH:(b + 1) * NCH],
                    )
                    row.append(t)
                xtb.append(row)

            for l in range(L):
                cA = cst.tile([1, AC], BF16, tag="cA")
                nc.sync.dma_start(out=cA[:], in_=d_cuattn[l, 0:1, :])
                uA = cst.tile([1, AC], BF16, tag="uA")
                nc.sync.dma_start(out=uA[:], in_=d_cuattn[l, 1:2, :])
                cF = cst.tile([1, FFL], BF16, tag="cF")
                nc.sync.dma_start(out=cF[:], in_=d_cufc[l, 0:1, :])
                uF = cst.tile([1, FFL], BF16, tag="uF")
                nc.sync.dma_start(out=uF[:], in_=d_cufc[l, 1:2, :])
                bout8_sb = cst.tile([128, KD], F32, tag="bout8")
                nc.sync.dma_start(out=bout8_sb[:], in_=d_bout8[l])
                bproj8_sb = cst.tile([128, KD], F32, tag="bproj8")
                nc.sync.dma_start(out=bproj8_sb[:], in_=d_bproj8[l])

                wa = []
                for k in range(KD):
                    t = wgt.tile([128, AC], BF16, tag=f"wa{k}")
                    nc.sync.dma_start(
                        out=t[:], in_=d_wattn[l, k * 128:(k + 1) * 128, :]
                    )
                    wa.append(t)
                wp = []
                for j in range(HPC):
                    t = wgt.tile([DH, D], BF16, tag=f"wp{j}")
                    nc.sync.dma_start(out=t[:], in_=d_wproj[l, j])
                    wp.append(t)

                # ---------- attention sublayer ----------
                for b in range(B):
                    attn_sublayer(b, l, xtb[b], wa, wp, cA, uA, bproj8_sb)
                all_reduce(l, 0)

                # FFN weights DMA overlaps the attention AllReduce
                wfc_sb = []
                for k in range(KD):
                    t = wgt.tile([128, FFL], BF16, tag=f"wbig{k}")
                    nc.sync.dma_start(
                        out=t[:], in_=d_wfc[l, k * 128:(k + 1) * 128, :]
                    )
                    wfc_sb.append(t)
                wout_sb = []
                for k in range(KF):
                    t = wgt.tile([128, D], BF16, tag=f"wo{k}")
                    nc.sync.dma_start(
                        out=t[:], in_=d_wout[l, k * 128:(k + 1) * 128, :]
                    )
                    wout_sb.append(t)

                # ---------- FFN sublayer ----------
                for b in range(B):
                    refresh_xt(xtb[b], l, 0, b)
                    ffn_sublayer(b, l, xtb[b], wfc_sb, wout_sb, cF, uF, bout8_sb)
                all_reduce(l, 1)

                if l < L - 1:
                    for b in range(B):
                        refresh_xt(xtb[b], l, 1, b)

            # ---------- final LN + vocab-sharded head ----------
            cH = cst.tile([1, VL], BF16, tag="cH")
            nc.sync.dma_start(out=cH[:], in_=d_cuhead[0:1, :])
            uH = cst.tile([1, VL], BF16, tag="uH")
            nc.sync.dma_start(out=uH[:], in_=d_cuhead[1:2, :])
            wh_sb = []
            for k in range(KD):
                t = wgt.tile([128, VL], BF16, tag=f"whd{k}")
                nc.sync.dma_start(out=t[:], in_=d_whead[k * 128:(k + 1) * 128, :])
                wh_sb.append(t)
            for b in range(B):
                refresh_xt(xtb[b], L - 1, 1, b)
                xb16, sq16 = make_x16(xtb[b], b)
                mneg16, sig16, rs16 = stats(xb16, sq16, b)
                rb_s = rb_bcast(rs16)
                for mi in range(VL // 128):
                    p = ps3.tile([128, NCH], F32, tag="mm")
                    for k in range(KD):
                        nc.tensor.matmul(
                            p[:], wh_sb[k][:, mi * 128:(mi + 1) * 128], xb16[k][:],
                            start=(k == 0), stop=False,
                        )
                    nc.tensor.matmul(
                        p[:], cH[:, mi * 128:(mi + 1) * 128], mneg16[:],
                        start=False, stop=False,
                    )
                    nc.tensor.matmul(
                        p[:], uH[:, mi * 128:(mi + 1) * 128], sig16[:],
                        start=False, stop=True,
                    )
                    lg = a3.tile([128, NCH], F32, tag="part1")
                    nc.vector.tensor_mul(lg[:], p[:], rb_s[:])
                    nc.sync.dma_start(
                        out=d_logits[mi * 128:(mi + 1) * 128, b * T:(b + 1) * T],
                        in_=lg[:, 0:T],
                    )

    nc.compile()
    return nc


def _get_nc(reps=1, use_cc=True):
    key = f"nc{reps}_{use_cc}"
    if key not in _CACHE:
        _CACHE[key] = _build_nc(reps, use_cc)
    return _CACHE[key]


# --------------------------------------------------------------------------
# numpy fallback (exact reference semantics for arbitrary edges)
# --------------------------------------------------------------------------

def _numpy_forward(inp):
    from scipy.special import erf

    def ln(x, g, b):
        m = x.mean(-1, keepdims=True)
        v = ((x - m) ** 2).mean(-1, keepdims=True)
        return (x - m) / np.sqrt(v + EPS) * g + b

    f32 = np.float32
    objs_e = np.asarray(inp["obj_emb_w"])[np.asarray(inp["objs"])]
    pe = np.asarray(inp["poss_emb_w"])[np.asarray(inp["poss"])]
    nfeat = np.concatenate([objs_e, pe[:, :NOBJ], pe[:, NOBJ:]], axis=-1)
    z = np.asarray(inp["tok_emb"])[np.asarray(inp["z_indices"])]
    x = np.concatenate([nfeat, z], axis=1) + np.asarray(inp["pos_emb"])[:, :T]
    x = x.reshape(N, D).astype(f32)
    src = np.asarray(inp["src"]).astype(np.int64)
    dst = np.asarray(inp["dst"]).astype(np.int64)
    for l in range(L):
        h = ln(x, inp["ln1_g"][l], inp["ln1_b"][l])
        Wh = (h @ np.asarray(inp["W_attn"][l])).reshape(N, H, DH)
        el = np.einsum("nhd,hd->nh", Wh, np.asarray(inp["a_l"][l]))
        er = np.einsum("nhd,hd->nh", Wh, np.asarray(inp["a_r"][l]))
        e = el[src] + er[dst]
        e = np.where(e >= 0, e, 0.2 * e)
        m = np.full((N, H), -np.inf, f32)
        np.maximum.at(m, dst, e)
        m[~np.isfinite(m)] = 0.0
        ex = np.exp(e - m[dst])
        s = np.zeros((N, H), f32)
        np.add.at(s, dst, ex)
        alpha = ex / s[dst]
        agg = np.zeros((N, H, DH), f32)
        np.add.at(agg, dst, alpha[:, :, None] * Wh[src])
        x = x + agg.reshape(N, D) @ np.asarray(inp["W_proj"][l]) \
            + np.asarray(inp["b_proj"][l])
        h2 = ln(x, inp["ln2_g"][l], inp["ln2_b"][l])
        ff = h2 @ np.asarray(inp["W_fc"][l]) + np.asarray(inp["b_fc"][l])
        ff = ff * 0.5 * (1.0 + erf(ff / np.sqrt(2.0)))
        x = x + ff @ np.asarray(inp["W_out"][l]) + np.asarray(inp["b_out"][l])
    x = ln(x, inp["lnf_g"], inp["lnf_b"])
    return (x @ np.asarray(inp["head_w"])).reshape(B, T, V).astype(f32)


# --------------------------------------------------------------------------
# public entry
# --------------------------------------------------------------------------

def _edges_are_block_diag(inp):
    src, dst = _block_diag_edges_np()
    s = np.asarray(inp["src"])
    d = np.asarray(inp["dst"])
    return (
        s.shape == src.shape
        and np.array_equal(s.astype(np.int64), src)
        and np.array_equal(d.astype(np.int64), dst)
    )


def _assemble(results):
    full = np.concatenate([results[c]["logits"] for c in range(NC)], axis=0)
    return np.ascontiguousarray(full.T).reshape(B, T, V)


def kernel(**inputs):
    if not _edges_are_block_diag(inputs):
        return _numpy_forward(inputs)
    from concourse import bass2jax

    in_maps = _host_inputs(inputs)
    results = bass2jax.run_bass_via_pjrt(_get_nc(), in_maps, n_cores=NC)
    return _assemble(results)


# --------------------------------------------------------------------------
# benchmarking (repeated execution, device-resident inputs)
# --------------------------------------------------------------------------

def _make_runner(nc):
    """Persistent jitted shard_map callable for nc (multi-core), mirroring
    bass2jax.run_bass_via_pjrt but reusable across calls."""
    import jax
    from jax.sharding import Mesh, PartitionSpec
    from jax.experimental.shard_map import shard_map
    from concourse import bass2jax, mybir as _mybir

    bass2jax.install_neuronx_cc_hook()
    partition_name = nc.partition_id_tensor.name if nc.partition_id_tensor else None
    in_names, out_names, out_avals, zero_outs = [], [], [], []
    for alloc in nc.m.functions[0].allocations:
        if not isinstance(alloc, _mybir.MemoryLocationSet):
            continue
        name = alloc.memorylocations[0].name
        if alloc.kind == "ExternalInput":
            if name != partition_name:
                in_names.append(name)
        elif alloc.kind == "ExternalOutput":
            shape = tuple(alloc.tensor_shape)
            dtype = _mybir.dt.np(alloc.dtype)
            out_names.append(name)
            out_avals.append(jax.core.ShapedArray(shape, dtype))
            zero_outs.append(np.zeros(shape, dtype))
    n_params = len(in_names)
    all_in_names = list(in_names) + list(out_names)
    if partition_name is not None:
        all_in_names.append(partition_name)

    def _body(*args):
        operands = list(args)
        if partition_name is not None:
            operands.append(bass2jax.partition_id_tensor())
        return tuple(
            bass2jax._bass_exec_p.bind(
                *operands,
                out_avals=tuple(out_avals),
                in_names=tuple(all_in_names),
                out_names=tuple(out_names),
                lowering_input_output_aliases=(),
                sim_require_finite=True,
                sim_require_nnan=True,
                nc=nc,
            )
        )

    devices = jax.devices()[:NC]
    mesh = Mesh(np.asarray(devices), ("core",))
    n_outs = len(out_names)
    in_specs = (PartitionSpec("core"),) * (n_params + n_outs)
    out_specs = (PartitionSpec("core"),) * n_outs
    donate = tuple(range(n_params, n_params + n_outs))
    fn = jax.jit(
        shard_map(_body, mesh=mesh, in_specs=in_specs, out_specs=out_specs,
                  check_rep=False),
        donate_argnums=donate, keep_unused=True,
    )
    return fn, in_names, out_names, zero_outs, mesh


def _timed_run(nc, in_maps, iters):
    """Median wall time (s) per execution with device-resident inputs."""
    import jax

    from jax.sharding import NamedSharding, PartitionSpec

    fn, in_names, out_names, zero_outs, mesh = _make_runner(nc)
    shard = NamedSharding(mesh, PartitionSpec("core"))
    concat_in = [
        np.concatenate([np.asarray(m[name]) for m in in_maps], axis=0)
        for name in in_names
    ]
    dev_in = [jax.device_put(a, shard) for a in concat_in]
    jax.block_until_ready(dev_in)

    def zeros():
        zs = [
            jax.device_put(
                np.zeros((NC * z.shape[0], *z.shape[1:]), z.dtype), shard
            )
            for z in zero_outs
        ]
        jax.block_until_ready(zs)
        return zs

    outs = fn(*dev_in, *zeros())  # warm-up/compile
    jax.block_until_ready(outs)
    times = []
    for _ in range(iters):
        zs = zeros()
        t0 = time.perf_counter()
        outs = fn(*dev_in, *zs)
        jax.block_until_ready(outs)
        times.append(time.perf_counter() - t0)
    return float(np.min(times)), outs, out_names


def bench(inputs, iters=16):
    """HW ns per network pass via reps-differential (cancels dispatch cost)."""
    in_maps = _host_inputs(inputs)
    t1, _, _ = _timed_run(_get_nc(1), in_maps, iters)
    t9, _, _ = _timed_run(_get_nc(9), in_maps, iters)
    print(f"  wall/iter reps1: {t1 * 1e6:.0f} us,  reps9: {t9 * 1e6:.0f} us")
    return max(t9 - t1, 0.0) / 8 * 1e9


# revision 31
# speedup vs baseline: 10.4152x; 10.4152x over previous
"""Trainium2 Bass kernel for nn_GAT_42786464203341.

8-way tensor parallel (Megatron-style) over one trn2 chip:
  - The GAT edges are block-diagonal fully-connected per sample, so message
    passing is dense per-sample attention with scores leaky(el[i] + er[j]),
    softmaxed over source i.
  - Activations feature-major (x^T: [D, nodes]); all matmuls bf16 (FWL
    weight loads, free-size-unconstrained); PSUM accumulates f32.
  - LayerNorm is folded into the weights: W~ = diag(g) W on the host, the
    -mu and +bias corrections enter each matmul as one rank-2 accumulation
    step (lhsT = [c; u], rhs = [-mu_row; sigma_row]), and the per-node
    1/sigma scale is applied at PSUM eviction (ACT per-partition scale for
    node-major outputs, DVE multiply by a broadcast row otherwise).  The
    explicit normalize / broadcast / affine chains of a direct LN are gone.
  - Attention is head-parallel (2 heads/core); W_proj row-sharded ->
    partial [D, nodes] -> AllReduce (fp16).  FFN column/row sharded ->
    AllReduce.  Head vocab-sharded; host concatenates the 8 logits slices.
"""

import time
from contextlib import ExitStack

import ml_dtypes
import numpy as np

import concourse.bass as bass
import concourse.tile as tile
from concourse import bacc, mybir
from concourse.masks import make_identity

F32 = mybir.dt.float32
F16 = mybir.dt.float16
BF16 = mybir.dt.bfloat16

B, T, NOBJ = 2, 265, 9
D, H, DH = 1536, 16, 96
V, PV, L, FF = 8192, 512, 3, 6144
N = B * T          # 530
NC = 8             # cores
HPC = H // NC      # heads per core
FFL = FF // NC     # 768
VL = V // NC       # 1024
NCH = T + 1        # 266 (col 265 of each chunk is zero padding)
NP = B * NCH       # 532
KD = D // 128      # 12
KF = FFL // 128    # 6
AC = 200           # folded attention weight cols: 192 Wh | 2 el | 2 er | 2 leaky | 2 pad
MT = [(0, 128), (128, 128), (256, 10)]   # node tiles per batch (start, size)
MT_REAL = [128, 128, 9]                  # non-pad rows per node tile
EPS = 1e-5

_CACHE = {}


# --------------------------------------------------------------------------
# host-side input prep
# --------------------------------------------------------------------------

def _block_diag_edges_np():
    base = np.arange(T)
    src = np.concatenate([g * T + np.repeat(base, T) for g in range(B)])
    dst = np.concatenate([g * T + np.tile(base, T) for g in range(B)])
    return src.astype(np.int64), dst.astype(np.int64)


def _host_inputs(inp):
    f32 = np.float32
    bf16 = ml_dtypes.bfloat16
    objs_e = np.asarray(inp["obj_emb_w"])[np.asarray(inp["objs"])]
    pe = np.asarray(inp["poss_emb_w"])[np.asarray(inp["poss"])]
    nfeat = np.concatenate([objs_e, pe[:, :NOBJ], pe[:, NOBJ:]], axis=-1)
    z = np.asarray(inp["tok_emb"])[np.asarray(inp["z_indices"])]
    x0 = np.concatenate([nfeat, z], axis=1) + np.asarray(inp["pos_emb"])[:, :T]
    x0 = x0.reshape(N, D).astype(f32)

    x0t = np.zeros((D, NP), f32)
    for b in range(B):
        x0t[:, b * NCH:b * NCH + T] = x0[b * T:(b + 1) * T].T

    W_attn = np.asarray(inp["W_attn"], f32)
    a_l = np.asarray(inp["a_l"], f32)
    a_r = np.asarray(inp["a_r"], f32)
    W_proj = np.asarray(inp["W_proj"], f32)
    W_fc = np.asarray(inp["W_fc"], f32)
    W_out = np.asarray(inp["W_out"], f32)
    head_w = np.asarray(inp["head_w"], f32)
    ln1_g = np.asarray(inp["ln1_g"], f32)
    ln1_b = np.asarray(inp["ln1_b"], f32)
    ln2_g = np.asarray(inp["ln2_g"], f32)
    ln2_b = np.asarray(inp["ln2_b"], f32)
    lnf_g = np.asarray(inp["lnf_g"], f32)
    lnf_b = np.asarray(inp["lnf_b"], f32)
    b_fc = np.asarray(inp["b_fc"], f32)

    def cols(vec, k_tiles):  # [D'] -> [128, k_tiles]
        return np.asarray(vec, f32).reshape(k_tiles, 128).T.copy()

    maps = []
    for c in range(NC):
        h0 = c * HPC
        # ---- attention: fold ln1 gain, a_l/a_r vectors -------------------
        wattn = np.zeros((L, D, AC), f32)
        uattn = np.zeros((L, AC), f32)
        for lx in range(L):
            for j in range(HPC):
                hg = h0 + j
                blk = W_attn[lx][:, hg * DH:(hg + 1) * DH]        # [D, DH]
                wel = blk @ a_l[lx, hg]                           # [D]
                wer = blk @ a_r[lx, hg]
                wattn[lx, :, j * DH:(j + 1) * DH] = ln1_g[lx][:, None] * blk
                wattn[lx, :, 192 + j] = ln1_g[lx] * wel
                wattn[lx, :, 194 + j] = ln1_g[lx] * wer
                uattn[lx, j * DH:(j + 1) * DH] = ln1_b[lx] @ blk
                uattn[lx, 192 + j] = ln1_b[lx] @ wel
                uattn[lx, 194 + j] = ln1_b[lx] @ wer
        cuattn = np.stack([wattn.sum(axis=1), uattn], axis=1)     # [L, 2, AC]

        wproj = np.stack(
            [W_proj[:, (h0 + j) * DH:(h0 + j + 1) * DH, :] for j in range(HPC)],
            axis=1,
        )                                                          # [L, HPC, DH, D]

        # ---- FFN: fold ln2 gain into W_fc, ln2 bias + b_fc into u -------
        fsl = slice(c * FFL, (c + 1) * FFL)
        wfc = ln2_g[:, :, None] * W_fc[:, :, fsl]                  # [L, D, FFL]
        ufc = np.einsum("ld,ldf->lf", ln2_b, W_fc[:, :, fsl]) + b_fc[:, fsl]
        cufc = np.stack([wfc.sum(axis=1), ufc], axis=1)            # [L, 2, FFL]

        wout = np.ascontiguousarray(W_out[:, fsl, :])              # [L, FFL, D]

        # ---- head: fold final ln ----------------------------------------
        vsl = slice(c * VL, (c + 1) * VL)
        whead = lnf_g[:, None] * head_w[:, vsl]                    # [D, VL]
        uhead = lnf_b @ head_w[:, vsl]
        cuhead = np.stack([whead.sum(axis=0), uhead], axis=0)      # [2, VL]

        maps.append({
            "x0t": x0t,
            "wattn": np.ascontiguousarray(wattn).astype(bf16),
            "cuattn": np.ascontiguousarray(cuattn).astype(bf16),
            "wproj": np.ascontiguousarray(wproj).astype(bf16),
            "wfc": np.ascontiguousarray(wfc).astype(bf16),
            "cufc": np.ascontiguousarray(cufc).astype(bf16),
            "wout": np.ascontiguousarray(wout).astype(bf16),
            "whead": np.ascontiguousarray(whead).astype(bf16),
            "cuhead": np.ascontiguousarray(cuhead).astype(bf16),
            "ones_col": np.ones((128, 1), bf16),
            "ones_row": np.ones((1, 128), bf16),
            "bout8_l": np.stack([cols(np.asarray(inp["b_out"], f32)[lx] / NC, KD)
                                 for lx in range(L)]),
            "bproj8_l": np.stack([cols(np.asarray(inp["b_proj"], f32)[lx] / NC, KD)
                                  for lx in range(L)]),
        })
    return maps


# --------------------------------------------------------------------------
# device program
# --------------------------------------------------------------------------

def _build_nc(reps=1, use_cc=True, nobias=True):
    nc = bacc.Bacc("TRN2", target_bir_lowering=False, debug=False, num_devices=NC)

    d_x0t = nc.declare_dram_parameter("x0t", [D, NP], F32, isOutput=False)
    d_wattn = nc.declare_dram_parameter("wattn", [L, D, AC], BF16, isOutput=False)
    d_cuattn = nc.declare_dram_parameter("cuattn", [L, 2, AC], BF16, isOutput=False)
    d_wproj = nc.declare_dram_parameter("wproj", [L, HPC, DH, D], BF16, isOutput=False)
    d_wfc = nc.declare_dram_parameter("wfc", [L, D, FFL], BF16, isOutput=False)
    d_cufc = nc.declare_dram_parameter("cufc", [L, 2, FFL], BF16, isOutput=False)
    d_wout = nc.declare_dram_parameter("wout", [L, FFL, D], BF16, isOutput=False)
    d_whead = nc.declare_dram_parameter("whead", [D, VL], BF16, isOutput=False)
    d_cuhead = nc.declare_dram_parameter("cuhead", [2, VL], BF16, isOutput=False)
    d_ones_col = nc.declare_dram_parameter("ones_col", [128, 1], BF16, isOutput=False)
    d_ones_row = nc.declare_dram_parameter("ones_row", [1, 128], BF16, isOutput=False)
    d_bout8 = nc.declare_dram_parameter("bout8_l", [L, 128, KD], F32, isOutput=False)
    d_bproj8 = nc.declare_dram_parameter("bproj8_l", [L, 128, KD], F32, isOutput=False)
    d_logits = nc.declare_dram_parameter("logits", [VL, N], F32, isOutput=True)

    ar_in, ar_out = {}, {}
    for l in range(L):
        for s in range(2):
            for b in range(B):
                ar_in[l, s, b] = nc.dram_tensor(f"arin_{l}_{s}_{b}", [D, T], F16)
                ar_out[l, s, b] = nc.dram_tensor(
                    f"arout_{l}_{s}_{b}", [D, T], F16, addr_space="Shared"
                )

    AF = mybir.ActivationFunctionType
    ALU = mybir.AluOpType

    with tile.TileContext(nc) as tc, ExitStack() as ctx:
        res = ctx.enter_context(tc.tile_pool(name="res", bufs=1))
        cst = ctx.enter_context(tc.tile_pool(name="cst", bufs=2))
        a1 = ctx.enter_context(tc.tile_pool(name="a1", bufs=1))
        a2 = ctx.enter_context(tc.tile_pool(name="a2", bufs=2))
        a3 = ctx.enter_context(tc.tile_pool(name="a3", bufs=3))
        wgt = ctx.enter_context(tc.tile_pool(name="wgt", bufs=1))
        ps2 = ctx.enter_context(tc.tile_pool(name="ps2", bufs=2, space="PSUM"))
        ps3 = ctx.enter_context(tc.tile_pool(name="ps3", bufs=3, space="PSUM"))

        ones_col = res.tile([128, 1], BF16, tag="ones_col")
        nc.sync.dma_start(out=ones_col[:], in_=d_ones_col[:])
        ones_row = res.tile([1, 128], BF16, tag="ones_row")
        nc.sync.dma_start(out=ones_row[:], in_=d_ones_row[:])
        ident = res.tile([128, 128], BF16, tag="ident")
        make_identity(nc, ident[:])
        identF = res.tile([128, 128], F32, tag="identF")
        make_identity(nc, identF[:])
        eps_col = res.tile([1, 1], F32, tag="eps")
        nc.vector.memset(eps_col[:], EPS)

        def stats(xb16, sq16, btag):
            """feature-major LN stats from bf16 x tiles.

            Returns (mneg16 [1, NCH] = -mu, sig16 [1, NCH] = sigma,
                     rs16 [1, NCH] = 1/sigma), all bf16."""
            p_sums = ps2.tile([1, NCH], F32, tag="row")
            for k in range(KD):
                nc.tensor.matmul(
                    p_sums[:], ones_col[:], xb16[k][:],
                    start=(k == 0), stop=(k == KD - 1),
                )
            p_sqs = ps2.tile([1, NCH], F32, tag="row")
            for k in range(KD):
                nc.tensor.matmul(
                    p_sqs[:], ones_col[:], sq16[k][:],
                    start=(k == 0), stop=(k == KD - 1),
                )
            mneg16 = a1.tile([1, NCH], BF16, name=f"mneg{btag}", tag=f"mneg{btag}")
            with nc.allow_low_precision("bf16 stats"):
                nc.vector.tensor_scalar(
                    mneg16[:], p_sums[:], -1.0 / D, None, ALU.mult
                )
            m_row = a1.tile([1, NCH], F32, tag="m_row")
            nc.vector.tensor_scalar(m_row[:], p_sums[:], 1.0 / D, None, ALU.mult)
            ms = a1.tile([1, NCH], F32, tag="ms_row")
            nc.scalar.activation(ms[:], m_row[:], AF.Square)
            var = a1.tile([1, NCH], F32, tag="var_row")
            nc.vector.scalar_tensor_tensor(
                var[:], p_sqs[:], 1.0 / D, ms[:], ALU.mult, ALU.subtract
            )
            std = a1.tile([1, NCH], F32, tag="std_row")
            nc.scalar.activation(std[:], var[:], AF.Sqrt, bias=eps_col[:])
            if nobias:
                sig16 = None
            else:
                sig16 = a1.tile([1, NCH], BF16, name=f"sig{btag}", tag=f"sig{btag}")
                nc.scalar.copy(sig16[:], std[:])
            rs16 = a1.tile([1, NCH], BF16, name=f"rs16{btag}", tag=f"rs16{btag}")
            with nc.allow_low_precision("bf16 stats"):
                nc.vector.reciprocal(rs16[:], std[:])
            return mneg16, sig16, rs16

        def make_x16(xb, btag):
            """bf16 copies of x plus bf16 squares."""
            xb16, sq16 = [], []
            for k in range(KD):
                t = a1.tile([128, NCH], BF16, name=f"x16{btag}_{k}", tag=f"x16{btag}_{k}")
                nc.scalar.copy(t[:], xb[k][:])
                xb16.append(t)
            for k in range(KD):
                t = a2.tile([128, NCH], BF16, tag=f"sq{k % 4}")
                nc.scalar.activation(t[:], xb[k][:], AF.Square)
                sq16.append(t)
            return xb16, sq16

        def rb_bcast(rs16):
            """broadcast 1/sigma row -> [128, NCH] f32 tile."""
            p_rb = ps3.tile([128, NCH], F32, tag="bc")
            nc.tensor.matmul(p_rb[:], ones_row[:], rs16[:], start=True, stop=True)
            rb_s = a1.tile([128, NCH], F32, tag="rb_s")
            nc.scalar.copy(rb_s[:], p_rb[:])
            return rb_s

        def partial_out(b, psum, b8_sb, mi, dram):
            """part = psum (+ b/8) in fp16; DMA into this batch's AR buffer."""
            part = a3.tile([128, NCH], F16, tag="part")
            bias = 0.0 if b8_sb is None else b8_sb[:, mi:mi + 1]
            with nc.allow_low_precision("fp16 allreduce payload"):
                nc.vector.tensor_scalar(
                    part[:], psum[:], bias, None, ALU.add
                )
            nc.sync.dma_start(
                out=dram[mi * 128:(mi + 1) * 128, :],
                in_=part[:, 0:T],
            )

        def all_reduce(l, s, b):
            if use_cc:
                nc.gpsimd.collective_compute(
                    "AllReduce", ALU.add,
                    replica_groups=[list(range(NC))],
                    ins=[ar_in[l, s, b][:].opt()],
                    outs=[ar_out[l, s, b][:].opt()],
                )
            else:
                nc.gpsimd.dma_start(out=ar_out[l, s, b][:], in_=ar_in[l, s, b][:])

        def refresh_xt(xb, l, s, b):
            for k in range(KD):
                tmp = a3.tile([128, T], F16, tag="artmp")
                nc.gpsimd.dma_start(
                    out=tmp[:],
                    in_=ar_out[l, s, b][k * 128:(k + 1) * 128, :],
                )
                nc.vector.tensor_add(xb[k][:, 0:T], xb[k][:, 0:T], tmp[:])

        def attn_sublayer(b, l, xb, wa, wp, cA, uA, bproj8_sb):
            xb16, sq16 = make_x16(xb, b)
            mneg16, sig16, rs16 = stats(xb16, sq16, b)

            # per-node-tile 1/sigma columns for ACT eviction scale (must be f32)
            rs_cols = []
            for mi, (ms, msz) in enumerate(MT):
                pt = ps2.tile([128, 1], BF16, tag="row")
                nc.tensor.transpose(
                    pt[:msz, :], rs16[:, ms:ms + msz], ident[0:1, 0:1]
                )
                rc = a1.tile([128, 1], F32, tag=f"rsc{mi}")
                nc.scalar.copy(rc[:msz, :], pt[:msz, :])
                rs_cols.append(rc)

            whsb, escs = [], []
            for mi, (ms, msz) in enumerate(MT):
                p = ps3.tile([128, AC], F32, tag="mm")
                for k in range(KD):
                    nc.tensor.matmul(
                        p[:msz, :], xb16[k][:, ms:ms + msz], wa[k][:],
                        start=(k == 0), stop=False,
                    )
                nc.tensor.matmul(
                    p[:msz, :], mneg16[:, ms:ms + msz], cA[:],
                    start=False, stop=nobias,
                )
                if not nobias:
                    nc.tensor.matmul(
                        p[:msz, :], sig16[:, ms:ms + msz], uA[:],
                        start=False, stop=True,
                    )
                w = a1.tile([128, 192], BF16, tag=f"whsb{mi}_{b}")
                nc.scalar.activation(
                    w[:msz, :], p[:msz, 0:192], AF.Copy,
                    scale=rs_cols[mi][:msz, :],
                )
                # esc cols: 0:2 el, 2:4 er, 4:6 leaky el (all f32, rs-scaled)
                esc = a1.tile([128, 8], F32, tag=f"esc{mi}_{b}")
                nc.scalar.activation(
                    esc[:msz, 0:4], p[:msz, 192:196], AF.Copy,
                    scale=rs_cols[mi][:msz, :],
                )
                nc.vector.tensor_scalar(
                    esc[:msz, 4:6], esc[:msz, 0:2], 0.2, None, ALU.mult
                )
                whsb.append(w)
                escs.append(esc)

            erow = [
                a1.tile([1, NCH], BF16, name=f"er{j}_{b}", tag=f"er{j}_{b}")
                for j in range(HPC)
            ]
            for mi, (ms, msz) in enumerate(MT):
                for j in range(HPC):
                    pt = ps2.tile([1, 128], F32, tag="row")
                    nc.tensor.transpose(
                        pt[:, :msz], escs[mi][:msz, 2 + j:3 + j],
                        identF[:msz, :msz],
                    )
                    with nc.allow_low_precision("bf16 scores"):
                        nc.scalar.copy(erow[j][:, ms:ms + msz], pt[:, :msz])

            aggt = []
            for j in range(HPC):
                p_er = ps3.tile([128, NCH], F32, tag="bc")
                nc.tensor.matmul(
                    p_er[:], ones_row[:], erow[j][:], start=True, stop=True
                )
                e_tiles = []
                for mi in range(3):
                    rsz = MT_REAL[mi]
                    e1 = a2.tile([128, NCH], BF16, tag=f"e{mi}")
                    nc.scalar.activation(
                        e1[:rsz, :], p_er[:rsz, :], AF.Exp,
                        bias=escs[mi][:rsz, j:j + 1],
                    )
                    e2 = a1.tile([128, NCH], BF16, tag="e2")
                    nc.scalar.activation(
                        e2[:rsz, :], p_er[:rsz, :], AF.Exp, scale=0.2,
                        bias=escs[mi][:rsz, 4 + j:5 + j],
                    )
                    with nc.allow_low_precision("bf16 scores"):
                        nc.vector.tensor_max(e1[:rsz, :], e1[:rsz, :], e2[:rsz, :])
                    e_tiles.append(e1)
                p_s = ps2.tile([1, NCH], F32, tag="row")
                for mi in range(3):
                    rsz = MT_REAL[mi]
                    nc.tensor.matmul(
                        p_s[:], ones_col[:rsz, :], e_tiles[mi][:rsz, :],
                        start=(mi == 0), stop=(mi == 2),
                    )
                r16 = a1.tile([1, NCH], BF16, tag="r16")
                with nc.allow_low_precision("bf16 softmax"):
                    nc.vector.reciprocal(r16[:], p_s[:])
                p_rb2 = ps3.tile([DH, NCH], F32, tag="bc")
                nc.tensor.matmul(
                    p_rb2[:], ones_row[:, :DH], r16[:], start=True, stop=True
                )
                rb_sb = a1.tile([DH, NCH], F32, tag="rb_sb")
                nc.scalar.copy(rb_sb[:], p_rb2[:])
                p_agg = ps3.tile([DH, NCH], F32, tag="mm")
                for mi in range(3):
                    rsz = MT_REAL[mi]
                    nc.tensor.matmul(
                        p_agg[:],
                        whsb[mi][:rsz, j * DH:(j + 1) * DH],
                        e_tiles[mi][:rsz, :],
                        start=(mi == 0), stop=(mi == 2),
                    )
                at = a1.tile([DH, NCH], BF16, tag=f"aggt{j}_{b}")
                with nc.allow_low_precision("bf16 agg"):
                    nc.vector.tensor_mul(at[:], p_agg[:], rb_sb[:])
                aggt.append(at)

            for mi in range(KD):
                p = ps3.tile([128, NCH], F32, tag="mm")
                for j in range(HPC):
                    nc.tensor.matmul(
                        p[:], wp[j][:, mi * 128:(mi + 1) * 128], aggt[j][:],
                        start=(j == 0), stop=(j == HPC - 1),
                    )
                partial_out(b, p, bproj8_sb, mi, ar_in[l, 0, b])

        def ffn_sublayer(b, l, xb, wfc_sb, wout_sb, cF, uF, bout8_sb):
            xb16, sq16 = make_x16(xb, b)
            mneg16, sig16, rs16 = stats(xb16, sq16, b)
            rb_s = rb_bcast(rs16)

            g_tiles = []
            for mi in range(KF):
                p = ps3.tile([128, NCH], F32, tag="mm")
                for k in range(KD):
                    nc.tensor.matmul(
                        p[:], wfc_sb[k][:, mi * 128:(mi + 1) * 128], xb16[k][:],
                        start=(k == 0), stop=False,
                    )
                nc.tensor.matmul(
                    p[:], cF[:, mi * 128:(mi + 1) * 128], mneg16[:],
                    start=False, stop=nobias,
                )
                if not nobias:
                    nc.tensor.matmul(
                        p[:], uF[:, mi * 128:(mi + 1) * 128], sig16[:],
                        start=False, stop=True,
                    )
                tmp = a2.tile([128, NCH], F32, tag="fftmp")
                nc.vector.tensor_mul(tmp[:], p[:], rb_s[:])
                g = a2.tile([128, NCH], BF16, tag=f"g{mi}")
                nc.scalar.activation(g[:], tmp[:], AF.Gelu)
                g_tiles.append(g)
            for mi in range(KD):
                p = ps3.tile([128, NCH], F32, tag="mm")
                for k in range(KF):
                    nc.tensor.matmul(
                        p[:], wout_sb[k][:, mi * 128:(mi + 1) * 128],
                        g_tiles[k][:],
                        start=(k == 0), stop=(k == KF - 1),
                    )
                partial_out(b, p, bout8_sb, mi, ar_in[l, 1, b])

        for _rep in range(reps):
            xtb = []
            for b in range(B):
                row = []
                for k in range(KD):
                    t = res.tile([128, NCH], F32, name=f"xt{b}_{k}", tag=f"xt{b}_{k}")
                    nc.sync.dma_start(
                        out=t[:],
                        in_=d_x0t[k * 128:(k + 1) * 128, b * NCH:(b + 1) * NCH],
                    )
                    row.append(t)
                xtb.append(row)

            for l in range(L):
                cA = cst.tile([1, AC], BF16, tag="cA")
                nc.sync.dma_start(out=cA[:], in_=d_cuattn[l, 0:1, :])
                cF = cst.tile([1, FFL], BF16, tag="cF")
                nc.sync.dma_start(out=cF[:], in_=d_cufc[l, 0:1, :])
                if nobias:
                    uA = uF = bout8_sb = bproj8_sb = None
                else:
                    uA = cst.tile([1, AC], BF16, tag="uA")
                    nc.sync.dma_start(out=uA[:], in_=d_cuattn[l, 1:2, :])
                    uF = cst.tile([1, FFL], BF16, tag="uF")
                    nc.sync.dma_start(out=uF[:], in_=d_cufc[l, 1:2, :])
                    bout8_sb = cst.tile([128, KD], F32, tag="bout8")
                    nc.sync.dma_start(out=bout8_sb[:], in_=d_bout8[l])
                    bproj8_sb = cst.tile([128, KD], F32, tag="bproj8")
                    nc.sync.dma_start(out=bproj8_sb[:], in_=d_bproj8[l])

                wa = []
                for k in range(KD):
                    t = wgt.tile([128, AC], BF16, tag=f"wa{k}")
                    nc.sync.dma_start(
                        out=t[:], in_=d_wattn[l, k * 128:(k + 1) * 128, :]
                    )
                    wa.append(t)
                wp = []
                for j in range(HPC):
                    t = wgt.tile([DH, D], BF16, tag=f"wp{j}")
                    nc.sync.dma_start(out=t[:], in_=d_wproj[l, j])
                    wp.append(t)
                # FFN weights DMA early: overlaps attention compute + ARs
                wfc_sb = []
                for k in range(KD):
                    t = wgt.tile([128, FFL], BF16, tag=f"wbig{k}")
                    nc.sync.dma_start(
                        out=t[:], in_=d_wfc[l, k * 128:(k + 1) * 128, :]
                    )
                    wfc_sb.append(t)
                wout_sb = []
                for k in range(KF):
                    t = wgt.tile([128, D], BF16, tag=f"wo{k}")
                    nc.sync.dma_start(
                        out=t[:], in_=d_wout[l, k * 128:(k + 1) * 128, :]
                    )
                    wout_sb.append(t)

                # ---------- attention sublayer (per-batch AR pipelining) ----
                for b in range(B):
                    attn_sublayer(b, l, xtb[b], wa, wp, cA, uA, bproj8_sb)
                    all_reduce(l, 0, b)

                # ---------- FFN sublayer ----------
                for b in range(B):
                    refresh_xt(xtb[b], l, 0, b)
                    ffn_sublayer(b, l, xtb[b], wfc_sb, wout_sb, cF, uF, bout8_sb)
                    all_reduce(l, 1, b)

                if l < L - 1:
                    for b in range(B):
                        refresh_xt(xtb[b], l, 1, b)

            # ---------- final LN + vocab-sharded head ----------
            cH = cst.tile([1, VL], BF16, tag="cH")
            nc.sync.dma_start(out=cH[:], in_=d_cuhead[0:1, :])
            if nobias:
                uH = None
            else:
                uH = cst.tile([1, VL], BF16, tag="uH")
                nc.sync.dma_start(out=uH[:], in_=d_cuhead[1:2, :])
            wh_sb = []
            for k in range(KD):
                t = wgt.tile([128, VL], BF16, tag=f"whd{k}")
                nc.sync.dma_start(out=t[:], in_=d_whead[k * 128:(k + 1) * 128, :])
                wh_sb.append(t)
            for b in range(B):
                refresh_xt(xtb[b], L - 1, 1, b)
                xb16, sq16 = make_x16(xtb[b], b)
                mneg16, sig16, rs16 = stats(xb16, sq16, b)
                rb_s = rb_bcast(rs16)
                for mi in range(VL // 128):
                    p = ps3.tile([128, NCH], F32, tag="mm")
                    for k in range(KD):
                        nc.tensor.matmul(
                            p[:], wh_sb[k][:, mi * 128:(mi + 1) * 128], xb16[k][:],
                            start=(k == 0), stop=False,
                        )
                    nc.tensor.matmul(
                        p[:], cH[:, mi * 128:(mi + 1) * 128], mneg16[:],
                        start=False, stop=nobias,
                    )
                    if not nobias:
                        nc.tensor.matmul(
                            p[:], uH[:, mi * 128:(mi + 1) * 128], sig16[:],
                            start=False, stop=True,
                        )
                    lg = a3.tile([128, NCH], F32, tag="part1")
                    nc.vector.tensor_mul(lg[:], p[:], rb_s[:])
                    nc.sync.dma_start(
                        out=d_logits[mi * 128:(mi + 1) * 128, b * T:(b + 1) * T],
                        in_=lg[:, 0:T],
                    )

    nc.compile()
    return nc


def _get_nc(reps=1, use_cc=True, nobias=True):
    key = f"nc{reps}_{use_cc}_{nobias}"
    if key not in _CACHE:
        _CACHE[key] = _build_nc(reps, use_cc, nobias)
    return _CACHE[key]


def _all_biases_zero(inp):
    return all(
        float(np.abs(np.asarray(inp[k])).max()) == 0.0
        for k in ("ln1_b", "ln2_b", "lnf_b", "b_proj", "b_fc", "b_out")
    )


# --------------------------------------------------------------------------
# numpy fallback (exact reference semantics for arbitrary edges)
# --------------------------------------------------------------------------

def _numpy_forward(inp):
    from scipy.special import erf

    def ln(x, g, b):
        m = x.mean(-1, keepdims=True)
        v = ((x - m) ** 2).mean(-1, keepdims=True)
        return (x - m) / np.sqrt(v + EPS) * g + b

    f32 = np.float32
    objs_e = np.asarray(inp["obj_emb_w"])[np.asarray(inp["objs"])]
    pe = np.asarray(inp["poss_emb_w"])[np.asarray(inp["poss"])]
    nfeat = np.concatenate([objs_e, pe[:, :NOBJ], pe[:, NOBJ:]], axis=-1)
    z = np.asarray(inp["tok_emb"])[np.asarray(inp["z_indices"])]
    x = np.concatenate([nfeat, z], axis=1) + np.asarray(inp["pos_emb"])[:, :T]
    x = x.reshape(N, D).astype(f32)
    src = np.asarray(inp["src"]).astype(np.int64)
    dst = np.asarray(inp["dst"]).astype(np.int64)
    for l in range(L):
        h = ln(x, inp["ln1_g"][l], inp["ln1_b"][l])
        Wh = (h @ np.asarray(inp["W_attn"][l])).reshape(N, H, DH)
        el = np.einsum("nhd,hd->nh", Wh, np.asarray(inp["a_l"][l]))
        er = np.einsum("nhd,hd->nh", Wh, np.asarray(inp["a_r"][l]))
        e = el[src] + er[dst]
        e = np.where(e >= 0, e, 0.2 * e)
        m = np.full((N, H), -np.inf, f32)
        np.maximum.at(m, dst, e)
        m[~np.isfinite(m)] = 0.0
        ex = np.exp(e - m[dst])
        s = np.zeros((N, H), f32)
        np.add.at(s, dst, ex)
        alpha = ex / s[dst]
        agg = np.zeros((N, H, DH), f32)
        np.add.at(agg, dst, alpha[:, :, None] * Wh[src])
        x = x + agg.reshape(N, D) @ np.asarray(inp["W_proj"][l]) \
            + np.asarray(inp["b_proj"][l])
        h2 = ln(x, inp["ln2_g"][l], inp["ln2_b"][l])
        ff = h2 @ np.asarray(inp["W_fc"][l]) + np.asarray(inp["b_fc"][l])
        ff = ff * 0.5 * (1.0 + erf(ff / np.sqrt(2.0)))
        x = x + ff @ np.asarray(inp["W_out"][l]) + np.asarray(inp["b_out"][l])
    x = ln(x, inp["lnf_g"], inp["lnf_b"])
    return (x @ np.asarray(inp["head_w"])).reshape(B, T, V).astype(f32)


# --------------------------------------------------------------------------
# public entry
# --------------------------------------------------------------------------

def _edges_are_block_diag(inp):
    src, dst = _block_diag_edges_np()
    s = np.asarray(inp["src"])
    d = np.asarray(inp["dst"])
    return (
        s.shape == src.shape
        and np.array_equal(s.astype(np.int64), src)
        and np.array_equal(d.astype(np.int64), dst)
    )


def _assemble(results):
    full = np.concatenate([results[c]["logits"] for c in range(NC)], axis=0)
    return np.ascontiguousarray(full.T).reshape(B, T, V)


def kernel(**inputs):
    if not _edges_are_block_diag(inputs):
        return _numpy_forward(inputs)
    from concourse import bass2jax

    in_maps = _host_inputs(inputs)
    nb = _all_biases_zero(inputs)
    results = bass2jax.run_bass_via_pjrt(
        _get_nc(nobias=nb), in_maps, n_cores=NC
    )
    return _assemble(results)


# --------------------------------------------------------------------------
# benchmarking (repeated execution, device-resident inputs)
# --------------------------------------------------------------------------

def _make_runner(nc):
    """Persistent jitted shard_map callable for nc (multi-core), mirroring
    bass2jax.run_bass_via_pjrt but reusable across calls."""
    import jax
    from jax.sharding import Mesh, PartitionSpec
    from jax.experimental.shard_map import shard_map
    from concourse import bass2jax, mybir as _mybir

    bass2jax.install_neuronx_cc_hook()
    partition_name = nc.partition_id_tensor.name if nc.partition_id_tensor else None
    in_names, out_names, out_avals, zero_outs = [], [], [], []
    for alloc in nc.m.functions[0].allocations:
        if not isinstance(alloc, _mybir.MemoryLocationSet):
            continue
        name = alloc.memorylocations[0].name
        if alloc.kind == "ExternalInput":
            if name != partition_name:
                in_names.append(name)
        elif alloc.kind == "ExternalOutput":
            shape = tuple(alloc.tensor_shape)
            dtype = _mybir.dt.np(alloc.dtype)
            out_names.append(name)
            out_avals.append(jax.core.ShapedArray(shape, dtype))
            zero_outs.append(np.zeros(shape, dtype))
    n_params = len(in_names)
    all_in_names = list(in_names) + list(out_names)
    if partition_name is not None:
        all_in_names.append(partition_name)

    def _body(*args):
        operands = list(args)
        if partition_name is not None:
            operands.append(bass2jax.partition_id_tensor())
        return tuple(
            bass2jax._bass_exec_p.bind(
                *operands,
                out_avals=tuple(out_avals),
                in_names=tuple(all_in_names),
                out_names=tuple(out_names),
                lowering_input_output_aliases=(),
                sim_require_finite=True,
                sim_require_nnan=True,
                nc=nc,
            )
        )

    devices = jax.devices()[:NC]
    mesh = Mesh(np.asarray(devices), ("core",))
    n_outs = len(out_names)
    in_specs = (PartitionSpec("core"),) * (n_params + n_outs)
    out_specs = (PartitionSpec("core"),) * n_outs
    donate = tuple(range(n_params, n_params + n_outs))
    fn = jax.jit(
        shard_map(_body, mesh=mesh, in_specs=in_specs, out_specs=out_specs,
                  check_rep=False),
        donate_argnums=donate, keep_unused=True,
    )
    return fn, in_names, out_names, zero_outs, mesh


def _timed_run(nc, in_maps, iters):
    """Median wall time (s) per execution with device-resident inputs."""
    import jax

    from jax.sharding import NamedSharding, PartitionSpec

    fn, in_names, out_names, zero_outs, mesh = _make_runner(nc)
    shard = NamedSharding(mesh, PartitionSpec("core"))
    concat_in = [
        np.concatenate([np.asarray(m[name]) for m in in_maps], axis=0)
        for name in in_names
    ]
    dev_in = [jax.device_put(a, shard) for a in concat_in]
    jax.block_until_ready(dev_in)

    def zeros():
        zs = [
            jax.device_put(
                np.zeros((NC * z.shape[0], *z.shape[1:]), z.dtype), shard
            )
            for z in zero_outs
        ]
        jax.block_until_ready(zs)
        return zs

    outs = fn(*dev_in, *zeros())  # warm-up/compile
    jax.block_until_ready(outs)
    times = []
    for _ in range(iters):
        zs = zeros()
        t0 = time.perf_counter()
        outs = fn(*dev_in, *zs)
        jax.block_until_ready(outs)
        times.append(time.perf_counter() - t0)
    return float(np.min(times)), outs, out_names


def bench(inputs, iters=16):
    """HW ns per network pass via reps-differential (cancels dispatch cost)."""
    in_maps = _host_inputs(inputs)
    t1, _, _ = _timed_run(_get_nc(1), in_maps, iters)
    t9, _, _ = _timed_run(_get_nc(9), in_maps, iters)
    print(f"  wall/iter reps1: {t1 * 1e6:.0f} us,  reps9: {t9 * 1e6:.0f} us")
    return max(t9 - t1, 0.0) / 8 * 1e9
